# revision 11
# baseline (speedup 1.0000x reference)
"""Trainium2 Bass kernel for nn_MixedOp_35562329211102.

Computes FM[b,c] = expm( sum_o weights[o] * logm( W[o,c]^T x[b,c] W[o,c] ) )
for x: [256,16,64,64] SPD, W: [6,16,64,32], weights: [6] (simplex).

Algorithm (matmul/elementwise only, no eigendecomposition):
  logm via a "W-only inverse-scaling" iteration: A = Y/theta, W0 = A,
  W_{j+1} = W_j * q_j(W_j)^2 with q_j(w) = a_j + b_j w chosen so the
  spectrum [1.7e-4, 0.96] is driven into [0.38, 1].  log(A) is then a
  fixed linear combination (global minimax fit, sup err ~5e-4) of the
  intermediates {I, W_j, G_j = W_j q_j(W_j), W_f, W_f^2, W_f^3, W_f^4}.
  expm via scaling-squaring: X = M/8, degree-6 Taylor (Paterson-
  Stockmeyer), then 3 squarings.

Execution: 4 matrices are batched per matmul instruction by building a
block-diagonal 128x128 stationary operand ("slab") holding b_j*W for 4
matrices; the affine identity term (a_j*I) is folded into the PSUM
eviction as a fused scalar_tensor_tensor:
  G      = a*W + slab(b*W) @ W      (slab build: 4 scaled copies)
  W_next = a*G + slab(b*W) @ G
This cuts the matmul instruction count ~4x vs per-matrix 32x32 matmuls
(the baseline was Tensor-sequencer issue-bound at ~34ns/instruction).

Sharding: data-parallel over batch B across 8 cores (32 batches/core).
Host-side pre/post permutes give 2KB+ DMA descriptors.
"""

import numpy as np

import concourse.bass as bass
from concourse import bacc
import concourse.mybir as mybir
from concourse.bass import AP
from concourse.tile import TileContext

FP = mybir.dt.float32
AOP = mybir.AluOpType

THETA = 9.0
LOGTHETA = 2.1972245773
ITERS = [
    (1.97108588, -1.13452036),
    (1.92678581, -1.0597322),
    (1.92678489, -1.05973169),
    (1.92678489, -1.05973169),
    (1.92678489, -1.05973169),
    (1.92678489, -1.05973169),
]
NIT = len(ITERS)
COEF = {
    'one': -10.50386520,
    'W0': 2.69748291, 'G0': -1.48453522,
    'W1': 3.00410138, 'G1': -1.52398907,
    'W2': 3.03186360, 'G2': -1.53628015,
    'W3': 3.03982436, 'G3': -1.54172997,
    'W4': 3.04361117, 'G4': -1.54522990,
    'W5': 3.04648630, 'G5': -1.54683948,
    'Wf': 6.88600636, 'P2': -7.62809900, 'P3': 5.13819165, 'P4': -1.40122234,
}
EXPC = [1.0, 1.0, 0.5, 1.0 / 6, 1.0 / 24, 1.0 / 120, 1.0 / 720, 1.0 / 5040,
        1.0 / 40320]

C, O, D, DIN = 16, 6, 32, 64
NCORES = 8

WT_KINDS = [f'W{j}' for j in range(NIT)] + ['Wf'] \
    + [f'G{j}' for j in range(NIT)] + ['P2', 'P3', 'P4']
WT_NCOL = len(WT_KINDS) * O


def host_wtab(weights: np.ndarray) -> np.ndarray:
    """[128, WT_NCOL] per-partition scalar table: w[o]/8 * coef (W0 also /theta)."""
    w8 = weights.astype(np.float64) / 8.0
    cols = []
    for k in WT_KINDS:
        s = COEF[k] / THETA if k == 'W0' else COEF[k]
        cols.append(w8 * s)
    row = np.concatenate(cols)
    return np.tile(row[None, :], (128, 1)).astype(np.float32)


def host_idt() -> np.ndarray:
    """[128, 32]: 4 stacked 32x32 identities."""
    return np.tile(np.eye(D, dtype=np.float32), (4, 1))


def host_x(x_core: np.ndarray, nchunk: int, bchunk: int) -> np.ndarray:
    """[b_loc,C,64,64] -> [nchunk, 8cp, 128(c2,p), 512(b,j)] (2KB/partition DMA)."""
    b_loc = x_core.shape[0]
    xh = x_core.reshape(nchunk, bchunk, 8, 2, DIN, DIN)
    xh = np.ascontiguousarray(xh.transpose(0, 2, 3, 4, 1, 5))
    return xh.reshape(nchunk, 8, 128, bchunk * DIN)


def host_w(W: np.ndarray) -> np.ndarray:
    """[6,16,64,32] -> [8cp, 128(e,p), 192(o,j)]."""
    wh = W.reshape(O, 8, 2, DIN, D).transpose(1, 2, 3, 0, 4)
    return np.ascontiguousarray(wh).reshape(8, 128, O * D)


def host_out(res: np.ndarray, nchunk: int, bchunk: int) -> np.ndarray:
    """[nchunk, 128, 1024] -> [b_loc, C, 32, 32]."""
    o = res.reshape(nchunk, 4, D, 4, bchunk, D).transpose(0, 4, 3, 1, 2, 5)
    return np.ascontiguousarray(o).reshape(nchunk * bchunk, C, D, D)


def build_nc(b_loc=32, bchunk=8, replicate=1):
    nchunk = b_loc // bchunk
    nb = bchunk * D          # 256: per-(o,c) stage2 N
    ncols = 4 * bchunk * D   # 1024: wave tile width (128 matrices)
    nblk = 4 * bchunk        # 32: 32x32 col-blocks (slabs) per wave tile

    nc = bacc.Bacc("TRN2")
    x = nc.dram_tensor("x", [nchunk, 8, 128, bchunk * DIN], FP,
                       kind="ExternalInput")
    Wt = nc.dram_tensor("W", [8, 128, O * D], FP, kind="ExternalInput")
    wtab_d = nc.dram_tensor("wtab", [128, WT_NCOL], FP, kind="ExternalInput")
    idt_d = nc.dram_tensor("idt", [128, D], FP, kind="ExternalInput")
    out = nc.dram_tensor("out", [nchunk, 128, ncols], FP, kind="ExternalOutput")

    with TileContext(nc) as tc, (
        tc.tile_pool(name="consts", bufs=1)) as consts, (
        tc.tile_pool(name="xp", bufs=3)) as xp, (
        tc.tile_pool(name="vp", bufs=2)) as vp, (
        tc.tile_pool(name="wog", bufs=12)) as wogp, (
        tc.tile_pool(name="arena", bufs=3)) as arp, (
        tc.tile_pool(name="gp", bufs=3)) as gp, (
        tc.tile_pool(name="ct", bufs=5)) as ctp, (
        tc.tile_pool(name="outp", bufs=2)) as outp, (
        tc.tile_pool(name="xaccp", bufs=7)) as xaccp, (
        tc.tile_pool(name="s1ps", bufs=1, space="PSUM")) as s1psp, (
        tc.tile_pool(name="s2ps", bufs=2, space="PSUM")) as s2psp, (
        tc.tile_pool(name="wkps", bufs=2, space="PSUM")) as wkps:

        # ---- constants ----
        w1t = []
        for cp in range(8):
            t = consts.tile([128, O * D], FP, tag=f"w1_{cp}")
            nc.sync.dma_start(t[:, :], Wt[cp])
            w1t.append(t)
        wtab = consts.tile([128, WT_NCOL], FP, tag="wtab", name="wtab")
        nc.sync.dma_start(wtab[:, :], wtab_d[:, :])
        idt = consts.tile([128, D], FP, tag="idt", name="idt")
        nc.sync.dma_start(idt[:, :], idt_d[:, :])
        cid3 = consts.tile([128, D], FP, tag="cid3")
        nc.vector.tensor_scalar_mul(cid3[:, :], idt[:, :], float(EXPC[3]))

        def wap(kind, o):
            i = WT_KINDS.index(kind) * O + o
            return wtab[:, i:i + 1]

        def idt_bc(t):
            return t[:, :].unsqueeze(1).broadcast_to([128, nblk, D])

        def blk(ap):
            return ap.rearrange("p (n j) -> p n j", n=nblk)

        # -- slab helpers: arena holds 32 slabs of 128x128 (blockdiag x4) --
        # Off-diagonal zeros are persistent: only diag-block columns are
        # ever written (scatter), so one memset per arena buffer suffices.
        ar_tiles = [arp.tile([128, nblk * 128], FP, tag="arena", name="arena")
                    for _ in range(3)]
        for t in ar_tiles:
            nc.gpsimd.memset(t[:, :], 0.0)
        ar_idx = [0]

        def next_arena():
            t = ar_tiles[ar_idx[0] % 3]
            ar_idx[0] += 1
            return t

        SC_ENGINES = [nc.scalar, nc.scalar, nc.vector, nc.gpsimd]

        def scatter(ar, src_tile, scale):
            """ar slabs <- blockdiag(scale * src), 4 partition-block copies."""
            for b4 in range(4):
                sl = slice(32 * b4, 32 * (b4 + 1))
                dst = ar[sl, :].rearrange("p (s k) -> p s k", k=128)[
                    :, :, 32 * b4:32 * (b4 + 1)]
                src = src_tile[sl, :].rearrange("p (s k) -> p s k", k=D)
                eng = SC_ENGINES[b4]
                if eng is nc.scalar:
                    eng.mul(dst, src, float(scale))
                else:
                    eng.tensor_scalar_mul(dst, src, float(scale))

        def slab_mm(ps, ar, rhs_tile):
            """ps[:, s*32:(s+1)*32] = slab_s @ rhs[:, s*32:(s+1)*32], all s."""
            for s in range(nblk):
                nc.tensor.matmul(ps[:, s * D:(s + 1) * D],
                                 ar[:, s * 128:(s + 1) * 128],
                                 rhs_tile[:, s * D:(s + 1) * D],
                                 start=True, stop=True)

        def vg(i):
            return nc.vector if i % 2 == 0 else nc.gpsimd

        for _rep in range(replicate):
          for ch in range(nchunk):
            wog = [None] * O
            xacc = [xaccp.tile([128, ncols], FP, tag="xacc", name="xacc")
                    for _ in range(O)]

            # ===== phase A: BiMap  Y[b,o,c] = W^T x W =====
            for q in range(4):
                vt = vp.tile([128, 2 * O * nb], FP, tag="v", name="v")
                xts = {}
                for cp in (2 * q, 2 * q + 1):
                    xt = xp.tile([128, bchunk * DIN], FP, tag="xt", name="xt")
                    nc.sync.dma_start(xt[:, :], x[ch, cp])
                    xts[cp] = xt
                for cp in (2 * q, 2 * q + 1):
                    e = cp % 2
                    xt = xts[cp]
                    for bb in range(bchunk):
                        ps1 = s1psp.tile([128, O * D], FP, tag="s1", name="s1")
                        xsl = xt[:, bb * DIN:(bb + 1) * DIN]
                        nc.tensor.matmul(ps1[0:64, :], xsl[0:64, :],
                                         w1t[cp][0:64, :],
                                         tile_position=(0, 0))
                        nc.tensor.matmul(ps1[64:128, :], xsl[64:128, :],
                                         w1t[cp][64:128, :],
                                         tile_position=(64, 64))
                        # scatter V into o-major layout
                        src = ps1[:, :].rearrange("p (o j) -> p o j", o=O)
                        va = vt[:, :]
                        dst = AP(va.tensor,
                                 va.offset + e * O * nb + bb * D,
                                 [list(va.ap[0]), [nb, O], [1, D]])
                        nc.vector.tensor_copy(dst, src)
                for o in range(O):
                    if q == 0:
                        wog[o] = wogp.tile([128, ncols], FP, tag="wog",
                                           name="wog")
                    ps2 = s2psp.tile([128, nb], FP, tag="s2", name="s2")
                    for cp in (2 * q, 2 * q + 1):
                        e = cp % 2
                        for par in range(2):
                            r = 2 * e + par
                            nc.tensor.matmul(
                                ps2[r * D:(r + 1) * D, :],
                                w1t[cp][par * 64:(par + 1) * 64,
                                        o * D:(o + 1) * D],
                                vt[par * 64:(par + 1) * 64,
                                   e * O * nb + o * nb:
                                   e * O * nb + (o + 1) * nb],
                                tile_position=(par * 64, r * D))
                    # evacuate Y -> W0 (x 1/theta); W0 term of the fit
                    nc.scalar.mul(wog[o][:, q * nb:(q + 1) * nb],
                                  ps2[:, :], 1.0 / THETA)
                    nc.scalar.mul(xacc[o][:, q * nb:(q + 1) * nb],
                                  ps2[:, :], wap('W0', o))

            # ===== phase B: log iterations (o-pairs for PE/DVE overlap) =====
            wcur = list(wog)
            for j in range(NIT):
                a, b = ITERS[j]
                for op in range(0, O, 2):
                    ars = []
                    for m in range(2):
                        ar = next_arena()
                        scatter(ar, wcur[op + m], b)
                        ars.append(ar)
                    gps_l = []
                    for m in range(2):
                        ps = wkps.tile([128, ncols], FP, tag="wk", name="wk")
                        slab_mm(ps, ars[m], wcur[op + m])
                        gps_l.append(ps)
                    gt_l = []
                    for m in range(2):
                        o = op + m
                        g = gp.tile([128, ncols], FP, tag="g", name="g")
                        nc.vector.scalar_tensor_tensor(
                            g[:, :], wcur[o][:, :], float(a), gps_l[m][:, :],
                            op0=AOP.mult, op1=AOP.add)
                        gt_l.append(g)
                    for m in range(2):
                        o = op + m
                        nc.vector.scalar_tensor_tensor(
                            xacc[o][:, :], gt_l[m][:, :], wap(f'G{j}', o),
                            xacc[o][:, :], op0=AOP.mult, op1=AOP.add)
                    kind = f'W{j + 1}' if j + 1 < NIT else 'Wf'
                    for m in range(2):
                        o = op + m
                        ps = wkps.tile([128, ncols], FP, tag="wk", name="wk")
                        slab_mm(ps, ars[m], gt_l[m])
                        wnew = wogp.tile([128, ncols], FP, tag="wog",
                                         name="wog")
                        nc.vector.scalar_tensor_tensor(
                            wnew[:, :], gt_l[m][:, :], float(a), ps[:, :],
                            op0=AOP.mult, op1=AOP.add)
                        nc.vector.scalar_tensor_tensor(
                            xacc[o][:, :], wnew[:, :], wap(kind, o),
                            xacc[o][:, :], op0=AOP.mult, op1=AOP.add)
                        wcur[o] = wnew

            # tail powers of Wf: P2 = Wf^2, P3 = Wf^3, P4 = Wf^4
            for op in range(0, O, 2):
                for m in range(2):
                    o = op + m
                    ar = next_arena()
                    scatter(ar, wcur[o], 1.0)
                    p2ps = wkps.tile([128, ncols], FP, tag="wk", name="wk")
                    slab_mm(p2ps, ar, wcur[o])
                    p2 = gp.tile([128, ncols], FP, tag="g", name="g")
                    nc.scalar.copy(p2[:, :], p2ps[:, :])
                    nc.vector.scalar_tensor_tensor(
                        xacc[o][:, :], p2[:, :], wap('P2', o),
                        xacc[o][:, :], op0=AOP.mult, op1=AOP.add)
                    p3ps = wkps.tile([128, ncols], FP, tag="wk", name="wk")
                    slab_mm(p3ps, ar, p2)
                    p3 = gp.tile([128, ncols], FP, tag="g", name="g")
                    nc.scalar.copy(p3[:, :], p3ps[:, :])
                    nc.vector.scalar_tensor_tensor(
                        xacc[o][:, :], p3[:, :], wap('P3', o),
                        xacc[o][:, :], op0=AOP.mult, op1=AOP.add)
                    p4ps = wkps.tile([128, ncols], FP, tag="wk", name="wk")
                    slab_mm(p4ps, ar, p3)
                    nc.vector.scalar_tensor_tensor(
                        xacc[o][:, :], p4ps[:, :], wap('P4', o),
                        xacc[o][:, :], op0=AOP.mult, op1=AOP.add)

            # ===== reduce partials + const, then phase C: expm =====
            r01 = ctp.tile([128, ncols], FP, tag="ctmp", name="ctmp")
            nc.vector.scalar_tensor_tensor(
                r01[:, :], xacc[0][:, :], 1.0, xacc[1][:, :],
                op0=AOP.mult, op1=AOP.add)
            r23 = ctp.tile([128, ncols], FP, tag="ctmp", name="ctmp")
            nc.gpsimd.tensor_add(r23[:, :], xacc[2][:, :], xacc[3][:, :])
            r45 = ctp.tile([128, ncols], FP, tag="ctmp", name="ctmp")
            nc.vector.scalar_tensor_tensor(
                r45[:, :], xacc[4][:, :], 1.0, xacc[5][:, :],
                op0=AOP.mult, op1=AOP.add)
            nc.gpsimd.tensor_add(r01[:, :], r23[:, :], r01[:, :])
            # xs = sum + ((one + logtheta)/8) * I
            xs = ctp.tile([128, ncols], FP, tag="ctmp", name="ctmp")
            nc.vector.scalar_tensor_tensor(
                r45[:, :], r01[:, :], 1.0, r45[:, :],
                op0=AOP.mult, op1=AOP.add)
            nc.vector.scalar_tensor_tensor(
                blk(xs[:, :]), idt_bc(idt),
                float((COEF['one'] + LOGTHETA) / 8.0),
                blk(r45[:, :]), op0=AOP.mult, op1=AOP.add)

            arx = next_arena()
            scatter(arx, xs, 1.0)
            x2ps = wkps.tile([128, ncols], FP, tag="wk", name="wk")
            slab_mm(x2ps, arx, xs)
            x2t = ctp.tile([128, ncols], FP, tag="ctmp", name="ctmp")
            nc.scalar.copy(x2t[:, :], x2ps[:, :])
            x3ps = wkps.tile([128, ncols], FP, tag="wk", name="wk")
            slab_mm(x3ps, arx, x2t)
            x3t = ctp.tile([128, ncols], FP, tag="ctmp", name="ctmp")
            nc.scalar.copy(x3t[:, :], x3ps[:, :])
            # h1 = c3 I + c4 xs + c5 x2 + c6 x3 ; plow = I + xs + c2 x2
            h1 = ctp.tile([128, ncols], FP, tag="ctmp", name="ctmp")
            nc.vector.scalar_tensor_tensor(
                blk(h1[:, :]), blk(xs[:, :]), float(EXPC[4]), idt_bc(cid3),
                op0=AOP.mult, op1=AOP.add)
            nc.vector.scalar_tensor_tensor(
                h1[:, :], x2t[:, :], float(EXPC[5]), h1[:, :],
                op0=AOP.mult, op1=AOP.add)
            nc.vector.scalar_tensor_tensor(
                h1[:, :], x3t[:, :], float(EXPC[6]), h1[:, :],
                op0=AOP.mult, op1=AOP.add)
            plow = ctp.tile([128, ncols], FP, tag="ctmp", name="ctmp")
            nc.vector.scalar_tensor_tensor(
                blk(plow[:, :]), blk(xs[:, :]), float(EXPC[1]), idt_bc(idt),
                op0=AOP.mult, op1=AOP.add)
            nc.vector.scalar_tensor_tensor(
                plow[:, :], x2t[:, :], float(EXPC[2]), plow[:, :],
                op0=AOP.mult, op1=AOP.add)
            arx3 = next_arena()
            scatter(arx3, x3t, 1.0)
            ppps = wkps.tile([128, ncols], FP, tag="wk", name="wk")
            slab_mm(ppps, arx3, h1)
            e0 = ctp.tile([128, ncols], FP, tag="ctmp", name="ctmp")
            nc.vector.scalar_tensor_tensor(
                e0[:, :], ppps[:, :], 1.0, plow[:, :],
                op0=AOP.mult, op1=AOP.add)
            cur = e0
            for sq in range(3):
                arq = next_arena()
                scatter(arq, cur, 1.0)
                eps_ = wkps.tile([128, ncols], FP, tag="wk", name="wk")
                slab_mm(eps_, arq, cur)
                if sq < 2:
                    nxt = ctp.tile([128, ncols], FP, tag="ctmp", name="ctmp")
                    nc.scalar.copy(nxt[:, :], eps_[:, :])
                    cur = nxt
                else:
                    outt = outp.tile([128, ncols], FP, tag="outt", name="outt")
                    nc.scalar.copy(outt[:, :], eps_[:, :])
            nc.sync.dma_start(out[ch], outt[:, :])
    return nc


_NC_CACHE = {}
NCHUNK = 4
BCHUNK = 8


def make_in_maps(x: np.ndarray, W: np.ndarray, weights: np.ndarray):
    B = x.shape[0]
    b_loc = B // NCORES
    wtab = host_wtab(np.asarray(weights))
    idt = host_idt()
    wh = host_w(np.asarray(W, dtype=np.float32))
    in_maps = []
    for i in range(NCORES):
        xc = np.asarray(x[i * b_loc:(i + 1) * b_loc], dtype=np.float32)
        in_maps.append({"x": host_x(xc, NCHUNK, BCHUNK), "W": wh,
                        "wtab": wtab, "idt": idt})
    return in_maps


def get_nc(b_loc):
    key = (b_loc,)
    if key not in _NC_CACHE:
        nc0 = build_nc(b_loc=b_loc, bchunk=BCHUNK)
        nc0.finalize()
        _NC_CACHE[key] = nc0
    return _NC_CACHE[key]


def kernel(x: np.ndarray, W: np.ndarray, weights: np.ndarray) -> np.ndarray:
    from concourse.bass_utils import run_bass_kernel_spmd
    B = x.shape[0]
    b_loc = B // NCORES
    nc = get_nc(b_loc)
    in_maps = make_in_maps(x, W, weights)
    res = run_bass_kernel_spmd(nc, in_maps, core_ids=list(range(NCORES)))
    return np.concatenate(
        [host_out(r["out"], NCHUNK, BCHUNK) for r in res.results], axis=0)


# revision 12
# speedup vs baseline: 2.6306x; 2.6306x over previous
"""Trainium2 Bass kernel for nn_MixedOp_35562329211102.

Computes FM[b,c] = expm( sum_o weights[o] * logm( W[o,c]^T x[b,c] W[o,c] ) )
for x: [256,16,64,64] SPD, W: [6,16,64,32], weights: [6] (simplex).

Algorithm (matmul/elementwise only, no eigendecomposition):
  logm via a monic degree-2 "inverse-scaling" iteration on V0 = -Y/theta:
    V_{j+1} = c * V_j + V_j^2     (one 32x32 matrix square per step)
  which is U_{j+1} = c*U_j - U_j^2 for U = -V: each step grows the small
  end of the spectrum by ~c=2.55 while keeping the top bounded.  log(Y)
  is then a linear combination (minimax fit on the actual Y spectrum
  [1.1e-3, 8.86], sup err 2.5e-3) of {I, V_0..V_9, Vf^2, Vf^3}: 11
  matrix products per logm (vs 15 for the deg-3 scheme).
  expm via scaling-squaring: X = M/8, degree-6 Taylor, 3 squarings.

Execution: 32x32 matmuls packed 4-up on the PE via tile_position (the
measured sweet spot: ~23ns busy / ~34ns issue per instruction; wider
128x128 stationaries cost ~214ns in weight reload).  Per-iteration
elementwise work: one fused PSUM-evict stt on DVE (V' = c*V + V^2),
and the fit-term accumulation with compile-time immediate coefficients
on the otherwise-idle Scalar (mul) + Pool (add) engines; the runtime
softmax weights enter only in a final 6-op weighted reduce.

Sharding: data-parallel over batch B across 8 cores (32 batches/core).
Host-side pre/post permutes give 2KB+ DMA descriptors.
"""

import numpy as np

import concourse.bass as bass
from concourse import bacc
import concourse.mybir as mybir
from concourse.bass import AP
from concourse.tile import TileContext

FP = mybir.dt.float32
AOP = mybir.AluOpType

# ---- deg-2 logm scheme (fit on y in [1.1e-3, 8.86], sup err 2.49e-3) ----
THETA = 5.436809816
CITER = 2.553429067
NIT = 9
# fit: log(y) ~ F_ONE*I + F_U[0]*U0 + ... + F_U[9]*U9 + F_P2*Uf^2 + F_P3*Uf^3
F_ONE = -8.31395629
F_U = [0.79977232, 0.56299771, 0.6193983, 0.59230569, 0.61343482,
       0.59379824, 0.61323371, 0.59357443, 0.61664669, 2.06722355]
F_P2 = -0.73505471
F_P3 = 0.08126438
# state sign trick: V_j = -U_j so V' = c*V + V@V (add-only stt).
# feature coefs in V: U_j = -V_j (odd sign), Vf^2 = Uf^2, Vf^3 = -Uf^3.
C_V = [-f for f in F_U]          # for V_0..V_9
C_P2 = F_P2
C_P3 = -F_P3

EXPC = [1.0, 1.0, 0.5, 1.0 / 6, 1.0 / 24, 1.0 / 120, 1.0 / 720]

C, O, D, DIN = 16, 6, 32, 64
NCORES = 8


def host_wtab(weights: np.ndarray) -> np.ndarray:
    """[128, O]: per-partition scalars w_o/8 for the final weighted reduce."""
    w8 = (weights.astype(np.float64) / 8.0).astype(np.float32)
    return np.tile(w8[None, :], (128, 1)).astype(np.float32)


def host_idt() -> np.ndarray:
    """[128, 32]: 4 stacked 32x32 identities."""
    return np.tile(np.eye(D, dtype=np.float32), (4, 1))


def host_x(x_core: np.ndarray, nchunk: int, bchunk: int) -> np.ndarray:
    """[b_loc,C,64,64] -> [nchunk, 8cp, 128(c2,p), 512(b,j)] (2KB/partition DMA)."""
    xh = x_core.reshape(nchunk, bchunk, 8, 2, DIN, DIN)
    xh = np.ascontiguousarray(xh.transpose(0, 2, 3, 4, 1, 5))
    return xh.reshape(nchunk, 8, 128, bchunk * DIN)


def host_w(W: np.ndarray) -> np.ndarray:
    """[6,16,64,32] -> [8cp, 128(e,p), 192(o,j)]."""
    wh = W.reshape(O, 8, 2, DIN, D).transpose(1, 2, 3, 0, 4)
    return np.ascontiguousarray(wh).reshape(8, 128, O * D)


def host_out(res: np.ndarray, nchunk: int, bchunk: int) -> np.ndarray:
    """[nchunk, 128, 1024] -> [b_loc, C, 32, 32]."""
    o = res.reshape(nchunk, 4, D, 4, bchunk, D).transpose(0, 4, 3, 1, 2, 5)
    return np.ascontiguousarray(o).reshape(nchunk * bchunk, C, D, D)


def build_nc(b_loc=32, bchunk=8, replicate=1):
    nchunk = b_loc // bchunk
    nb = bchunk * D          # 256: per-(o,c) stage2 N
    ncols = 4 * bchunk * D   # 1024: wave tile width (128 matrices)
    nblk = 4 * bchunk        # 32: 32x32 col-blocks per wave tile

    nc = bacc.Bacc("TRN2")
    x = nc.dram_tensor("x", [nchunk, 8, 128, bchunk * DIN], FP,
                       kind="ExternalInput")
    Wt = nc.dram_tensor("W", [8, 128, O * D], FP, kind="ExternalInput")
    wtab_d = nc.dram_tensor("wtab", [128, O], FP, kind="ExternalInput")
    idt_d = nc.dram_tensor("idt", [128, D], FP, kind="ExternalInput")
    out = nc.dram_tensor("out", [nchunk, 128, ncols], FP, kind="ExternalOutput")

    with TileContext(nc) as tc, (
        tc.tile_pool(name="consts", bufs=1)) as consts, (
        tc.tile_pool(name="xp", bufs=3)) as xp, (
        tc.tile_pool(name="vp", bufs=2)) as vp, (
        tc.tile_pool(name="wog", bufs=12)) as wogp, (
        tc.tile_pool(name="gp", bufs=6)) as gp, (
        tc.tile_pool(name="ct", bufs=7)) as ctp, (
        tc.tile_pool(name="outp", bufs=2)) as outp, (
        tc.tile_pool(name="xaccp", bufs=8)) as xaccp, (
        tc.tile_pool(name="s1ps", bufs=1, space="PSUM")) as s1psp, (
        tc.tile_pool(name="s2ps", bufs=2, space="PSUM")) as s2psp, (
        tc.tile_pool(name="wkps", bufs=2, space="PSUM")) as wkps:

        # ---- constants ----
        w1t = []
        for cp in range(8):
            t = consts.tile([128, O * D], FP, tag=f"w1_{cp}", name="w1")
            nc.sync.dma_start(t[:, :], Wt[cp])
            w1t.append(t)
        wtab = consts.tile([128, O], FP, tag="wtab", name="wtab")
        nc.sync.dma_start(wtab[:, :], wtab_d[:, :])
        idt = consts.tile([128, D], FP, tag="idt", name="idt")
        nc.sync.dma_start(idt[:, :], idt_d[:, :])
        cid3 = consts.tile([128, D], FP, tag="cid3", name="cid3")
        nc.vector.tensor_scalar_mul(cid3[:, :], idt[:, :], float(EXPC[3]))

        def wap(o):
            return wtab[:, o:o + 1]

        def idt_bc(t):
            return t[:, :].unsqueeze(1).broadcast_to([128, nblk, D])

        def blk(ap):
            return ap.rearrange("p (n j) -> p n j", n=nblk)

        def mmwave(dst, lhs, rhs):
            """128 matrices: 32x32 matmuls packed 4-up via PE tiling."""
            for cb in range(nblk):
                cs = slice(cb * D, (cb + 1) * D)
                for i in range(4):
                    sl = slice(i * D, (i + 1) * D)
                    nc.tensor.matmul(dst[sl, cs], lhs[sl, cs], rhs[sl, cs],
                                     start=True, stop=True,
                                     tile_position=(i * D, i * D))

        for _rep in range(replicate):
          for ch in range(nchunk):
            wog = [None] * O
            xacc = [None] * O

            # ===== phase A: BiMap  Y[b,o,c] = W^T x W;  V0 = -Y/theta =====
            for q in range(4):
                vt = vp.tile([128, 2 * O * nb], FP, tag="v", name="v")
                xts = {}
                for cp in (2 * q, 2 * q + 1):
                    xt = xp.tile([128, bchunk * DIN], FP, tag="xt", name="xt")
                    nc.sync.dma_start(xt[:, :], x[ch, cp])
                    xts[cp] = xt
                for cp in (2 * q, 2 * q + 1):
                    e = cp % 2
                    xt = xts[cp]
                    for bb in range(bchunk):
                        ps1 = s1psp.tile([128, O * D], FP, tag="s1", name="s1")
                        xsl = xt[:, bb * DIN:(bb + 1) * DIN]
                        nc.tensor.matmul(ps1[0:64, :], xsl[0:64, :],
                                         w1t[cp][0:64, :],
                                         tile_position=(0, 0))
                        nc.tensor.matmul(ps1[64:128, :], xsl[64:128, :],
                                         w1t[cp][64:128, :],
                                         tile_position=(64, 64))
                        src = ps1[:, :].rearrange("p (o j) -> p o j", o=O)
                        va = vt[:, :]
                        dst = AP(va.tensor,
                                 va.offset + e * O * nb + bb * D,
                                 [list(va.ap[0]), [nb, O], [1, D]])
                        nc.vector.tensor_copy(dst, src)
                for o in range(O):
                    if q == 0:
                        wog[o] = wogp.tile([128, ncols], FP, tag="wog",
                                           name="wog")
                        xacc[o] = xaccp.tile([128, ncols], FP, tag="xacc",
                                             name="xacc")
                    ps2 = s2psp.tile([128, nb], FP, tag="s2", name="s2")
                    for cp in (2 * q, 2 * q + 1):
                        e = cp % 2
                        for par in range(2):
                            r = 2 * e + par
                            nc.tensor.matmul(
                                ps2[r * D:(r + 1) * D, :],
                                w1t[cp][par * 64:(par + 1) * 64,
                                        o * D:(o + 1) * D],
                                vt[par * 64:(par + 1) * 64,
                                   e * O * nb + o * nb:
                                   e * O * nb + (o + 1) * nb],
                                tile_position=(par * 64, r * D))
                    # V0 = -Y/theta ; xacc = C_V[0] * V0
                    qs = slice(q * nb, (q + 1) * nb)
                    nc.scalar.mul(wog[o][:, qs], ps2[:, :], -1.0 / THETA)
                    nc.scalar.mul(xacc[o][:, qs], ps2[:, :],
                                  float(-C_V[0] / THETA))

            # ===== phase B: V' = c*V + V@V, accumulate C_V[j+1]*V' =====
            vcur = list(wog)
            for j in range(NIT):
                for op in range(0, O, 2):
                    ps_l = []
                    for m in range(2):
                        ps = wkps.tile([128, ncols], FP, tag="wk", name="wk")
                        mmwave(ps, vcur[op + m], vcur[op + m])
                        ps_l.append(ps)
                    for m in range(2):
                        o = op + m
                        vnew = wogp.tile([128, ncols], FP, tag="wog",
                                         name="wog")
                        nc.vector.scalar_tensor_tensor(
                            vnew[:, :], vcur[o][:, :], float(CITER),
                            ps_l[m][:, :], op0=AOP.mult, op1=AOP.add)
                        tmp = gp.tile([128, ncols], FP, tag="g", name="g")
                        nc.scalar.mul(tmp[:, :], vnew[:, :],
                                      float(C_V[j + 1]))
                        nc.gpsimd.tensor_add(xacc[o][:, :], xacc[o][:, :],
                                             tmp[:, :])
                        vcur[o] = vnew

            # tail: P2 = Vf^2, P3 = Vf^2 @ Vf
            for op in range(0, O, 2):
                p2_l = []
                for m in range(2):
                    o = op + m
                    ps = wkps.tile([128, ncols], FP, tag="wk", name="wk")
                    mmwave(ps, vcur[o], vcur[o])
                    p2 = gp.tile([128, ncols], FP, tag="g", name="g")
                    nc.scalar.copy(p2[:, :], ps[:, :])
                    tmp = gp.tile([128, ncols], FP, tag="g", name="g")
                    nc.scalar.mul(tmp[:, :], ps[:, :], float(C_P2))
                    nc.gpsimd.tensor_add(xacc[o][:, :], xacc[o][:, :],
                                         tmp[:, :])
                    p2_l.append(p2)
                for m in range(2):
                    o = op + m
                    ps = wkps.tile([128, ncols], FP, tag="wk", name="wk")
                    mmwave(ps, p2_l[m], vcur[o])
                    tmp = gp.tile([128, ncols], FP, tag="g", name="g")
                    nc.scalar.mul(tmp[:, :], ps[:, :], float(C_P3))
                    nc.gpsimd.tensor_add(xacc[o][:, :], xacc[o][:, :],
                                         tmp[:, :])

            # ===== weighted reduce:  M/8 = sum_o (w_o/8) xacc_o + cI =====
            racc = ctp.tile([128, ncols], FP, tag="ctmp", name="ctmp")
            nc.vector.tensor_scalar_mul(racc[:, :], xacc[0][:, :], wap(0))
            for o in range(1, O):
                nc.vector.scalar_tensor_tensor(
                    racc[:, :], xacc[o][:, :], wap(o), racc[:, :],
                    op0=AOP.mult, op1=AOP.add)
            xs = ctp.tile([128, ncols], FP, tag="ctmp", name="ctmp")
            nc.vector.scalar_tensor_tensor(
                blk(xs[:, :]), idt_bc(idt), float(F_ONE / 8.0),
                blk(racc[:, :]), op0=AOP.mult, op1=AOP.add)

            # ===== phase C: expm (deg-6 Taylor + 3 squarings) =====
            x2ps = wkps.tile([128, ncols], FP, tag="wk", name="wk")
            mmwave(x2ps, xs, xs)
            x2t = ctp.tile([128, ncols], FP, tag="ctmp", name="ctmp")
            nc.scalar.copy(x2t[:, :], x2ps[:, :])
            x3ps = wkps.tile([128, ncols], FP, tag="wk", name="wk")
            mmwave(x3ps, x2t, xs)
            x3t = ctp.tile([128, ncols], FP, tag="ctmp", name="ctmp")
            nc.scalar.copy(x3t[:, :], x3ps[:, :])
            h1 = ctp.tile([128, ncols], FP, tag="ctmp", name="ctmp")
            nc.vector.scalar_tensor_tensor(
                blk(h1[:, :]), blk(xs[:, :]), float(EXPC[4]), idt_bc(cid3),
                op0=AOP.mult, op1=AOP.add)
            nc.vector.scalar_tensor_tensor(
                h1[:, :], x2t[:, :], float(EXPC[5]), h1[:, :],
                op0=AOP.mult, op1=AOP.add)
            nc.vector.scalar_tensor_tensor(
                h1[:, :], x3t[:, :], float(EXPC[6]), h1[:, :],
                op0=AOP.mult, op1=AOP.add)
            plow = ctp.tile([128, ncols], FP, tag="ctmp", name="ctmp")
            nc.vector.scalar_tensor_tensor(
                blk(plow[:, :]), blk(xs[:, :]), float(EXPC[1]), idt_bc(idt),
                op0=AOP.mult, op1=AOP.add)
            nc.vector.scalar_tensor_tensor(
                plow[:, :], x2t[:, :], float(EXPC[2]), plow[:, :],
                op0=AOP.mult, op1=AOP.add)
            ppps = wkps.tile([128, ncols], FP, tag="wk", name="wk")
            mmwave(ppps, x3t, h1)
            e0 = ctp.tile([128, ncols], FP, tag="ctmp", name="ctmp")
            nc.vector.scalar_tensor_tensor(
                e0[:, :], ppps[:, :], 1.0, plow[:, :],
                op0=AOP.mult, op1=AOP.add)
            cur = e0
            for sq in range(3):
                eps_ = wkps.tile([128, ncols], FP, tag="wk", name="wk")
                mmwave(eps_, cur, cur)
                if sq < 2:
                    nxt = ctp.tile([128, ncols], FP, tag="ctmp", name="ctmp")
                    nc.scalar.copy(nxt[:, :], eps_[:, :])
                    cur = nxt
                else:
                    outt = outp.tile([128, ncols], FP, tag="outt", name="outt")
                    nc.scalar.copy(outt[:, :], eps_[:, :])
            nc.sync.dma_start(out[ch], outt[:, :])
    return nc


_NC_CACHE = {}
NCHUNK = 4
BCHUNK = 8


def make_in_maps(x: np.ndarray, W: np.ndarray, weights: np.ndarray):
    B = x.shape[0]
    b_loc = B // NCORES
    wtab = host_wtab(np.asarray(weights))
    idt = host_idt()
    wh = host_w(np.asarray(W, dtype=np.float32))
    in_maps = []
    for i in range(NCORES):
        xc = np.asarray(x[i * b_loc:(i + 1) * b_loc], dtype=np.float32)
        in_maps.append({"x": host_x(xc, NCHUNK, BCHUNK), "W": wh,
                        "wtab": wtab, "idt": idt})
    return in_maps


def get_nc(b_loc):
    key = (b_loc,)
    if key not in _NC_CACHE:
        nc0 = build_nc(b_loc=b_loc, bchunk=BCHUNK)
        nc0.finalize()
        _NC_CACHE[key] = nc0
    return _NC_CACHE[key]


def kernel(x: np.ndarray, W: np.ndarray, weights: np.ndarray) -> np.ndarray:
    from concourse.bass_utils import run_bass_kernel_spmd
    B = x.shape[0]
    b_loc = B // NCORES
    nc = get_nc(b_loc)
    in_maps = make_in_maps(x, W, weights)
    res = run_bass_kernel_spmd(nc, in_maps, core_ids=list(range(NCORES)))
    return np.concatenate(
        [host_out(r["out"], NCHUNK, BCHUNK) for r in res.results], axis=0)
